# revision 18
# baseline (speedup 1.0000x reference)
"""Locally-connected network (28x28 -> lc3x3 -> lc3x3 -> fc10) on 8 TRN2 cores.

The whole reference network is linear (two locally-connected layers + FC, no
activations), so on the host we fold it into a single affine map
    out[b, :] = x[b, :784] @ M + c          (M: [784, 10], c: [10])
computed in float64. The device kernel is a pure data-parallel, memory-bound
matmul. v2: everything the PE touches is bf16 (1 cycle/row vs fp32's 4, and
half the HBM traffic); each core streams its 1024-sample shard as 4 batch
chunks of 256 so PSUM accumulation starts after the first ~400KB instead of
after the full shard; chunk loads alternate between the two HWDGE rings
(sync / scalar); the folded M rides in chunk0's buffer so no matmul ever
needs a second DMA-lane wait; bias is added by the Vector engine (DVE) during
the PSUM->SBUF copy, keeping ScalarE (and its ACT_TABLE_LOAD preamble) out of
the kernel entirely.
"""

import numpy as np
from ml_dtypes import bfloat16

import concourse.bass as bass
import concourse.tile as tile
from concourse import bacc, mybir
from concourse.bass_utils import run_bass_kernel_spmd

N_CORES = 8
B = 8192
B_SHARD = B // N_CORES          # 1024
PIX = 784                       # 28*28
KP = 112                        # K-tile partition count; 784 = 7 * 112
NKT = PIX // KP                 # 7
CHUNKS = [128, 224, 224, 224, 224]  # batch chunk sizes (sum = B_SHARD);
# first chunk small so the PE starts ~1.5us earlier
NCHUNK = len(CHUNKS)
NOUT = 10


def _lc_dense(w, H, W_, oh, ow):
    """Dense [H*W_, oh*ow] matrix of one 3x3 locally-connected layer."""
    w = np.asarray(w, np.float64).reshape(oh, ow, 9)
    M = np.zeros((H * W_, oh * ow), np.float64)
    ox, oy = np.meshgrid(np.arange(oh), np.arange(ow), indexing="ij")
    col = (ox * ow + oy).ravel()
    for i in range(3):
        for j in range(3):
            row = ((ox + i) * W_ + (oy + j)).ravel()
            M[row, col] += w[:, :, i * 3 + j].ravel()
    return M


def _fold(w1, b1, w2, b2, fc_w, fc_b):
    W1 = _lc_dense(w1, 28, 28, 26, 26)          # [784, 676]
    W2 = _lc_dense(w2, 26, 26, 24, 24)          # [676, 576]
    fcw = np.asarray(fc_w, np.float64)          # [10, 576]
    M = W1 @ W2 @ fcw.T                         # [784, 10]
    c = (
        np.asarray(b1, np.float64).reshape(-1) @ W2
        + np.asarray(b2, np.float64).reshape(-1)
    ) @ fcw.T + np.asarray(fc_b, np.float64)    # [10]
    return M.astype(np.float32), c.astype(np.float32)


def _build_bass():
    nc = bacc.Bacc("TRN2", target_bir_lowering=False, debug=False)
    # chunk0 carries x chunk 0 in slots 0..6 plus the folded M in slot 7.
    xc = [
        nc.declare_dram_parameter(
            f"xc{i}",
            [KP, (NKT + 1) if i == 0 else NKT, CHUNKS[i]],
            mybir.dt.bfloat16,
            isOutput=False,
        )
        for i in range(NCHUNK)
    ]
    out = nc.declare_dram_parameter("out", [NOUT, B_SHARD], mybir.dt.float32, isOutput=True)

    with tile.TileContext(nc) as tc:
        with (
            tc.tile_pool(name="xp", bufs=NCHUNK) as xp,
            tc.tile_pool(name="pp", bufs=NCHUNK, space="PSUM") as pp,
            tc.tile_pool(name="op", bufs=1) as op,
        ):
            # x chunk loads spread across both HWDGE rings (sync / scalar)
            # plus GpSimd's SWDGE as a third queue — one queue alone feeds
            # only ~140 B/ns; three together reach ~300+.
            rings = [nc.sync, nc.scalar, nc.gpsimd, nc.sync, nc.scalar]
            xts = []
            for i in range(NCHUNK):
                t = xp.tile(
                    [KP, (NKT + 1) if i == 0 else NKT, CHUNKS[i]], mybir.dt.bfloat16
                )
                rings[i].dma_start(t[:], xc[i][:])
                xts.append(t)
            m_sb = xts[0]  # slot NKT holds M; cols 70:72 of it hold the fp32
            # bias for partitions 0..9 (bitcast below), so there is no
            # separate bias DMA (whose 4-byte descriptors cost ~1.5us of
            # HWDGE time and got hoisted ahead of the x loads).
            b_ap = m_sb[0:NOUT, NKT, 70:72].bitcast(mybir.dt.float32)

            # Absorb the chunk0 DMA-lane wait on DVE with a throwaway copy,
            # so the real bias-adds below wait only on the PE semaphore.
            scratch = op.tile([1, 1], mybir.dt.float32)
            nc.vector.tensor_copy(scratch[:], b_ap[0:1, 0:1])

            o = op.tile([NOUT, B_SHARD], mybir.dt.float32)
            off = 0
            for ch in range(NCHUNK):
                ps = pp.tile([NOUT, CHUNKS[ch]], mybir.dt.float32)
                for kt in range(NKT):
                    nc.tensor.matmul(
                        ps[:],
                        m_sb[:, NKT, kt * NOUT : (kt + 1) * NOUT],
                        xts[ch][:, kt, :],
                        start=(kt == 0),
                        stop=(kt == NKT - 1),
                    )
                nc.vector.tensor_scalar_add(
                    o[:, off : off + CHUNKS[ch]], ps[:], b_ap
                )
                off += CHUNKS[ch]
            nc.sync.dma_start(out[:], o[:])
    nc.finalize()
    return nc


def _run(inputs, trace=False, trace_cores=None):
    x = np.asarray(inputs["x"], np.float32)
    M, c = _fold(
        inputs["w1"], inputs["b1"], inputs["w2"], inputs["b2"],
        inputs["fc_w"], inputs["fc_b"],
    )
    # m header slot: m_sb[p, kt*10 + o] = M[kt*112 + p, o]; cols 70:72 of
    # partitions 0..9 hold the fp32 bias (device reads them via bitcast).
    mh = np.zeros((KP, CHUNKS[0]), bfloat16)
    mh[:, : NKT * NOUT] = (
        M.reshape(NKT, KP, NOUT).transpose(1, 0, 2).reshape(KP, NKT * NOUT)
    ).astype(bfloat16)
    mh[:NOUT, 70:72] = (
        c.reshape(NOUT, 1).astype(np.float32).view(np.uint16).view(bfloat16)
    )

    # xt[p, kt, b] = x[b, kt*112 + p] per shard, bf16
    xr = x.reshape(B, PIX).astype(bfloat16)
    in_maps = []
    for i in range(N_CORES):
        sh = (
            xr[i * B_SHARD : (i + 1) * B_SHARD]
            .reshape(B_SHARD, NKT, KP)
            .transpose(2, 1, 0)
        )  # [112, 7, 1024]
        im = {}
        off = 0
        for cidx in range(NCHUNK):
            blk = sh[:, :, off : off + CHUNKS[cidx]]  # [112, 7, CHUNKS[cidx]]
            off += CHUNKS[cidx]
            if cidx == 0:
                buf = np.empty((KP, NKT + 1, CHUNKS[0]), bfloat16)
                buf[:, :NKT, :] = blk
                buf[:, NKT, :] = mh
                im["xc0"] = buf
            else:
                im[f"xc{cidx}"] = np.ascontiguousarray(blk)
        in_maps.append(im)

    nc = _build_bass()
    res = run_bass_kernel_spmd(
        nc,
        in_maps,
        list(range(N_CORES)),
        trace=trace,
        trace_cores=trace_cores,
    )
    out = np.concatenate(
        [np.asarray(res.results[i]["out"]).T for i in range(N_CORES)], axis=0
    ).astype(np.float32)
    return out, res


def kernel(**inputs) -> np.ndarray:
    out, _ = _run(inputs, trace=False)
    return out
